# revision 1
# baseline (speedup 1.0000x reference)
"""Trainium2 Bass kernel for nn_ColorRenderer (SoftRas-style color renderer).

Algorithm (per pixel p, over faces f):
  winner(p) = argmax_f score(f,p),   score = min(BIG*w0, BIG*w1, BIG*w2, -depth)
  valid(p)  = maxscore > THRESH;     color(p) = winner's affine color eval.

Each of the 4 "banks" (BIG*w0, BIG*w1, BIG*w2, -depth) is an affine function
of pixel coords, evaluated on the TensorEngine as a K=9 bf16 matmul
(3-way bf16 split of each f64 coefficient; pixel coords recentered so they
are exact small integers in bf16).  The 4 banks are packed into 4 PE
row-groups (tile_position row tiling) and run concurrently.

Sharding: the host computes the global screen bbox of all projected faces
(everything outside is provably background) and deals bbox pixels
round-robin to the 8 cores.  Each core rasterizes all (padded) 1024 faces
for its pixels.  Host does only projection / coefficient prep (O(V+F)) and
final scatter of per-core pixel colors into the [1,3,256,256] frame.
"""

import numpy as np
import ml_dtypes

IMAGE_SIZE = 256
ORIG_SIZE = 512
DENOM_EPS = 1e-8

BIG = 1e14
THRESH = -5000.0
BAD = -3.0e30
NCORES = 8
FPAD = 1024
BLK = 512

bf16 = ml_dtypes.bfloat16

_PROGRAM_CACHE = {}


# ----------------------------------------------------------------------------
# Host-side math (projection, coefficients)
# ----------------------------------------------------------------------------

def _project_f32(vertices, K, R, t, dist_coeffs):
    """Faithful float32 replication of the reference projection."""
    f32 = np.float32
    EPS = f32(1e-9)
    v = np.einsum('bij,bvj->bvi', R.astype(f32), vertices.astype(f32)).astype(f32) + t.astype(f32)
    x, y, z = v[..., 0], v[..., 1], v[..., 2]
    x_ = (x / (z + EPS)).astype(f32)
    y_ = (y / (z + EPS)).astype(f32)
    r2 = (x_ * x_ + y_ * y_).astype(f32)
    d = dist_coeffs.astype(f32)
    k1 = d[:, 0:1]; k2 = d[:, 1:2]; p1 = d[:, 2:3]; p2 = d[:, 3:4]; k3 = d[:, 4:5]
    radial = (f32(1.0) + k1 * r2 + k2 * r2 ** 2 + k3 * r2 ** 3).astype(f32)
    x__ = (x_ * radial + f32(2.0) * p1 * x_ * y_ + p2 * (r2 + f32(2.0) * x_ * x_)).astype(f32)
    y__ = (y_ * radial + p1 * (r2 + f32(2.0) * y_ * y_) + f32(2.0) * p2 * x_ * y_).astype(f32)
    ones = np.ones_like(x__)
    uv = np.einsum('bij,bvj->bvi', K.astype(f32),
                   np.stack([x__, y__, ones], -1).astype(f32)).astype(f32)
    OS = f32(ORIG_SIZE)
    u = (f32(2.0) * (uv[..., 0] - OS / 2) / OS).astype(f32)
    vv = (f32(2.0) * ((OS - uv[..., 1]) - OS / 2) / OS).astype(f32)
    return np.stack([u, vv, z], -1).astype(f32)          # [B,V,3]


def _face_vertices_f32(verts, faces):
    f32 = np.float32
    IM = f32(IMAGE_SIZE)
    fv = verts[0][faces[0]]                               # [F,3,3]
    fv = fv * np.array([1.0, -1.0, 1.0], dtype=f32)
    fv = (fv * (IM / 2) + IM / 2).astype(f32)
    return fv


def _build_coeffs(fv):
    """Per-face f64 affine coefficients for w0,w1,w2,negdepth + ok mask."""
    f = fv.astype(np.float64)
    x0, y0, z0 = f[:, 0, 0], f[:, 0, 1], f[:, 0, 2]
    x1, y1, z1 = f[:, 1, 0], f[:, 1, 1], f[:, 1, 2]
    x2, y2, z2 = f[:, 2, 0], f[:, 2, 1], f[:, 2, 2]
    denom_f32 = ((fv[:, 1, 1] - fv[:, 2, 1]) * (fv[:, 0, 0] - fv[:, 2, 0])
                 + (fv[:, 2, 0] - fv[:, 1, 0]) * (fv[:, 0, 1] - fv[:, 2, 1])).astype(np.float32)
    ok = np.abs(denom_f32) > np.float32(DENOM_EPS)
    d = np.where(ok, (y1 - y2) * (x0 - x2) + (x2 - x1) * (y0 - y2), 1.0)
    a0 = (y1 - y2) / d; b0 = (x2 - x1) / d
    c0 = (-(y1 - y2) * x2 - (x2 - x1) * y2) / d
    a1 = (y2 - y0) / d; b1 = (x0 - x2) / d
    c1 = (-(y2 - y0) * x2 - (x0 - x2) * y2) / d
    a2 = (y0 - y1) / d; b2 = (x1 - x0) / d
    c2 = (-(y0 - y1) * x1 - (x1 - x0) * y1) / d
    and_ = -(a0 * z0 + a1 * z1 + a2 * z2)
    bnd = -(b0 * z0 + b1 * z1 + b2 * z2)
    cnd = -(c0 * z0 + c1 * z1 + c2 * z2)
    return dict(ok=ok, w0=(a0, b0, c0), w1=(a1, b1, c1), w2=(a2, b2, c2),
                nd=(and_, bnd, cnd))


def _split3_bf16(a):
    h = a.astype(bf16)
    r1 = a - h.astype(np.float64)
    m = r1.astype(bf16)
    l = (r1 - m.astype(np.float64)).astype(bf16)
    return h, m, l


def _bank_rows(a, b, c, sx, sy, mask_bad, bad_c):
    """9 bf16 coefficient rows for one bank (recentered at sx, sy)."""
    cc = c + a * sx + b * sy
    a = np.where(mask_bad, 0.0, a)
    b = np.where(mask_bad, 0.0, b)
    cc = np.where(mask_bad, bad_c, cc)
    ah, am, al = _split3_bf16(a)
    bh, bm, bl = _split3_bf16(b)
    ch, cm, cl = _split3_bf16(cc)
    return np.stack([ah, am, al, bh, bm, bl, ch, cm, cl], 0)   # [9, F] bf16


# ----------------------------------------------------------------------------
# Bass program
# ----------------------------------------------------------------------------

def _build_program(nch, fpad):
    STAGE = 4  # full pipeline (lower values were build-time debug bisection)
    import concourse.bacc as bacc
    import concourse.tile as tile
    import concourse.bass as bass
    from concourse import mybir
    from contextlib import ExitStack

    dt = mybir.dt
    op = mybir.AluOpType
    nc = bacc.Bacc("TRN2", target_bir_lowering=False, debug=False,
                   num_devices=NCORES)

    pixlhs = nc.dram_tensor("pixlhs", [128, nch * 128], dt.bfloat16, kind="ExternalInput")
    coefs = nc.dram_tensor("coefs", [128, fpad], dt.bfloat16, kind="ExternalInput")
    pxf = nc.dram_tensor("pxf", [128, nch], dt.float32, kind="ExternalInput")
    pyf = nc.dram_tensor("pyf", [128, nch], dt.float32, kind="ExternalInput")
    colco = nc.dram_tensor("colco", [fpad, 16], dt.float32, kind="ExternalInput")
    colout = nc.dram_tensor("colout", [3, 128, nch], dt.float32, kind="ExternalOutput")
    dbgmax = nc.dram_tensor("dbgmax", [128, nch], dt.float32, kind="ExternalOutput")
    dbgidx = nc.dram_tensor("dbgidx", [128, nch], dt.uint32, kind="ExternalOutput")

    nblk = fpad // BLK

    with tile.TileContext(nc) as tc, ExitStack() as ctx:
        const = ctx.enter_context(tc.tile_pool(name="const", bufs=1))
        psum = ctx.enter_context(tc.tile_pool(name="psum", bufs=2, space="PSUM"))
        work = ctx.enter_context(tc.tile_pool(name="work", bufs=4))
        scorep = ctx.enter_context(tc.tile_pool(name="scorep", bufs=2))
        accp = ctx.enter_context(tc.tile_pool(name="accp", bufs=1))

        pix_sb = const.tile([128, nch * 128], dt.bfloat16)
        nc.sync.dma_start(out=pix_sb[:], in_=pixlhs[:])
        coef_sb = const.tile([128, fpad], dt.bfloat16)
        nc.sync.dma_start(out=coef_sb[:], in_=coefs[:])
        pxf_sb = const.tile([128, nch], dt.float32)
        nc.sync.dma_start(out=pxf_sb[:], in_=pxf[:])
        pyf_sb = const.tile([128, nch], dt.float32)
        nc.sync.dma_start(out=pyf_sb[:], in_=pyf[:])

        maxall = accp.tile([128, nch], dt.float32)
        idx32 = accp.tile([128, nch], dt.uint32)
        gath = accp.tile([128, nch, 16], dt.float32)

        for j in range(nch):
            score = scorep.tile([128, fpad], dt.float32, tag="score")
            for b in range(nblk):
                banks = []
                for g in range(4):
                    pt = psum.tile([128, BLK], dt.float32, tag=f"bank{g}")
                    nc.tensor.matmul(
                        pt[:],
                        pix_sb[32 * g:32 * g + 9, j * 128:(j + 1) * 128],
                        coef_sb[32 * g:32 * g + 9, b * BLK:(b + 1) * BLK],
                        start=True, stop=True,
                        tile_position=(32 * g, 0),
                    )
                    banks.append(pt)
                # TT can read only one PSUM operand: ACT copies banks 0,2 to SBUF
                a_sb = work.tile([128, BLK], dt.float32, tag="a_sb")
                c_sb = work.tile([128, BLK], dt.float32, tag="c_sb")
                nc.scalar.copy(out=a_sb[:], in_=banks[0][:])
                nc.scalar.copy(out=c_sb[:], in_=banks[2][:])
                x = work.tile([128, BLK], dt.float32, tag="x")
                y = work.tile([128, BLK], dt.float32, tag="y")
                nc.vector.tensor_tensor(out=x[:], in0=a_sb[:], in1=banks[1][:], op=op.min)
                nc.vector.tensor_tensor(out=y[:], in0=c_sb[:], in1=banks[3][:], op=op.min)
                nc.vector.tensor_tensor(
                    out=score[:, b * BLK:(b + 1) * BLK],
                    in0=x[:], in1=y[:], op=op.min)
            cmx = work.tile([128, 1], dt.float32, tag="cmx")
            nc.vector.tensor_reduce(out=cmx[:], in_=score[:], axis=mybir.AxisListType.X, op=op.max)
            nc.scalar.copy(out=maxall[:, j:j + 1], in_=cmx[:])
            if STAGE >= 2:
                # broadcast chunk max to the 8-wide in_max format of max_index
                cm_ap = cmx[:, 0:1]
                cm8 = bass.AP(tensor=cm_ap.tensor, offset=cm_ap.offset,
                              ap=[cm_ap.ap[0], [0, 8]])
                idx8 = work.tile([128, 8], dt.uint32, tag="idx8")
                nc.vector.max_index(out=idx8[:], in_max=cm8, in_values=score[:])
                nc.vector.tensor_copy(out=idx32[:, j:j + 1], in_=idx8[:, 0:1])

        if STAGE < 2:
            nc.vector.memset(idx32[:], 0)
        # phase 2: gather per-pixel winner color coefficients, eval affine
        if STAGE >= 3:
            for j in range(nch):
                nc.gpsimd.indirect_dma_start(
                    out=gath[:, j, :], out_offset=None,
                    in_=colco[:],
                    in_offset=bass.IndirectOffsetOnAxis(ap=idx32[:, j:j + 1], axis=0),
                    bounds_check=fpad - 1, oob_is_err=False,
                )
        else:
            nc.vector.memset(gath[:], 0)
        vmask = work.tile([128, nch], dt.float32, tag="vmask")
        nc.vector.tensor_scalar(out=vmask[:], in0=maxall[:], scalar1=float(THRESH),
                                scalar2=None, op0=op.is_gt)
        nc.sync.dma_start(out=dbgmax[:], in_=maxall[:])
        nc.sync.dma_start(out=dbgidx[:], in_=idx32[:])
        for ch in range(3):
            m1 = work.tile([128, nch], dt.float32, tag="m1")
            m2 = work.tile([128, nch], dt.float32, tag="m2")
            cv = work.tile([128, nch], dt.float32, tag="cv")
            nc.vector.tensor_tensor(out=m1[:], in0=gath[:, :, 3 * ch + 0], in1=pxf_sb[:], op=op.mult)
            nc.vector.tensor_tensor(out=m2[:], in0=gath[:, :, 3 * ch + 1], in1=pyf_sb[:], op=op.mult)
            nc.vector.tensor_tensor(out=m1[:], in0=m1[:], in1=m2[:], op=op.add)
            nc.vector.tensor_tensor(out=m1[:], in0=m1[:], in1=gath[:, :, 3 * ch + 2], op=op.add)
            nc.vector.tensor_tensor(out=cv[:], in0=m1[:], in1=vmask[:], op=op.mult)
            nc.sync.dma_start(out=colout[ch], in_=cv[:])

    nc.compile()
    return nc


def _get_program(nch, fpad):
    key = (nch, fpad)
    if key not in _PROGRAM_CACHE:
        _PROGRAM_CACHE[key] = _build_program(nch, fpad)
    return _PROGRAM_CACHE[key]


# ----------------------------------------------------------------------------
# Host orchestration
# ----------------------------------------------------------------------------

def prepare(vertices, faces, textures, K, R, t, dist_coeffs):
    """All host-side prep.  Returns (nch, in_maps, scatter_info)."""
    verts = _project_f32(np.asarray(vertices), np.asarray(K), np.asarray(R),
                         np.asarray(t), np.asarray(dist_coeffs))
    fv = _face_vertices_f32(verts, np.asarray(faces))
    F = fv.shape[0]
    co = _build_coeffs(fv)
    bad = ~co['ok']
    tex = np.asarray(textures)[0].astype(np.float64)      # [F,3,C]

    # global bbox (+1px margin) of all face vertices
    xmin, xmax = fv[:, :, 0].min(), fv[:, :, 0].max()
    ymin, ymax = fv[:, :, 1].min(), fv[:, :, 1].max()
    c_lo = max(0, int(np.floor(xmin - 0.5)) - 1)
    c_hi = min(IMAGE_SIZE - 1, int(np.ceil(xmax - 0.5)) + 1)
    r_lo = max(0, int(np.floor(ymin - 0.5)) - 1)
    r_hi = min(IMAGE_SIZE - 1, int(np.ceil(ymax - 0.5)) + 1)
    if c_hi < c_lo or r_hi < r_lo:
        return None                                        # nothing visible

    ncols = c_hi - c_lo + 1
    nrows = r_hi - r_lo + 1
    G = nrows * ncols
    nch = max(1, -(-G // (NCORES * 128)))
    S = nch * 128

    # centered pixel coords must be bf16-exact small integers
    sx = np.floor((c_lo + c_hi) / 2) + 0.5
    sy = np.floor((r_lo + r_hi) / 2) + 0.5
    span = max(c_hi - c_lo, r_hi - r_lo) / 2 + 2
    assert span <= 192, "bbox too large for bf16-exact centered coords"

    fpad = max(BLK, -(-F // BLK) * BLK)
    pad = np.zeros(fpad - F, dtype=bool)
    badp = np.concatenate([bad, ~pad])                     # padded faces are bad

    def padded(a):
        return np.concatenate([a, np.zeros(fpad - F, dtype=np.float64)])

    # score banks (w scaled by BIG, negdepth unscaled)
    coefs = np.zeros((128, fpad), dtype=bf16)
    for g, name in enumerate(['w0', 'w1', 'w2']):
        a, b, c = (padded(v) * BIG for v in co[name])
        coefs[32 * g:32 * g + 9, :] = _bank_rows(a, b, c, sx, sy, badp, BAD)
    a, b, c = (padded(v) for v in co['nd'])
    coefs[96:96 + 9, :] = _bank_rows(a, b, c, sx, sy, badp, BAD)

    # color affine coefficients table [fpad, 16] f32 (recentered)
    colco = np.zeros((fpad, 16), dtype=np.float32)
    for ch in range(3):
        t0, t1, t2 = tex[:, 0, ch], tex[:, 1, ch], tex[:, 2, ch]
        A = padded(co['w0'][0] * t0 + co['w1'][0] * t1 + co['w2'][0] * t2)
        B = padded(co['w0'][1] * t0 + co['w1'][1] * t1 + co['w2'][1] * t2)
        C = padded(co['w0'][2] * t0 + co['w1'][2] * t1 + co['w2'][2] * t2)
        C = C + A * sx + B * sy
        colco[:, 3 * ch + 0] = A.astype(np.float32)
        colco[:, 3 * ch + 1] = B.astype(np.float32)
        colco[:, 3 * ch + 2] = C.astype(np.float32)

    # pixel lists per core: global bbox pixel g -> core g%8, slot g//8
    # slot s <-> (partition p, chunk j): s = p*nch + j
    in_maps = []
    rows_of = np.empty((NCORES, S), dtype=np.int32)
    cols_of = np.empty((NCORES, S), dtype=np.int32)
    real_of = np.empty((NCORES, S), dtype=bool)
    for k in range(NCORES):
        g = np.arange(S) * NCORES + k
        real = g < G
        gc = np.where(real, g, 0)
        rr = gc // ncols + r_lo
        cc = gc % ncols + c_lo
        rows_of[k] = rr; cols_of[k] = cc; real_of[k] = real
        pxc = (cc + 0.5) - sx                              # exact ints
        pyc = (rr + 0.5) - sy
        # s = p*nch + j ; lhsT column index = j*128 + p
        s = np.arange(S)
        p = s // nch
        j = s % nch
        colidx = j * 128 + p
        pixlhs = np.zeros((128, S), dtype=bf16)
        pxb = pxc.astype(bf16); pyb = pyc.astype(bf16)
        assert np.all(pxb.astype(np.float64) == pxc)
        assert np.all(pyb.astype(np.float64) == pyc)
        onerow = np.ones(S, dtype=bf16)
        rows9 = [pxb, pxb, pxb, pyb, pyb, pyb, onerow, onerow, onerow]
        for gg in range(4):
            for r in range(9):
                pixlhs[32 * gg + r, colidx] = rows9[r]
        pxf = np.zeros((128, nch), dtype=np.float32)
        pyf = np.zeros((128, nch), dtype=np.float32)
        pxf[p, j] = pxc.astype(np.float32)
        pyf[p, j] = pyc.astype(np.float32)
        in_maps.append(dict(pixlhs=pixlhs, coefs=coefs, pxf=pxf, pyf=pyf,
                            colco=colco))

    scatter = dict(rows_of=rows_of, cols_of=cols_of, real_of=real_of, nch=nch,
                   fpad=fpad)
    return nch, fpad, in_maps, scatter


def assemble(results, scatter):
    out = np.zeros((1, 3, IMAGE_SIZE, IMAGE_SIZE), dtype=np.float32)
    nch = scatter['nch']
    for k in range(NCORES):
        col = results[k]['colout']                         # [3, 128, nch]
        flat = col.reshape(3, 128 * nch)                   # slot s = p*nch+j
        real = scatter['real_of'][k]
        rr = scatter['rows_of'][k][real]
        cc = scatter['cols_of'][k][real]
        out[0, :, rr, cc] = flat[:, real].T
    return out


def kernel(**inputs):
    from concourse.bass_utils import run_bass_kernel_spmd

    prep = prepare(**inputs)
    if prep is None:
        return np.zeros((1, 3, IMAGE_SIZE, IMAGE_SIZE), dtype=np.float32)
    nch, fpad, in_maps, scatter = prep
    nc = _get_program(nch, fpad)
    res = run_bass_kernel_spmd(nc, in_maps, core_ids=list(range(NCORES)))
    return assemble(res.results, scatter)


if __name__ == "__main__":
    pass



# revision 19
# speedup vs baseline: 2.0306x; 2.0306x over previous
"""Trainium2 Bass kernel for nn_ColorRenderer (SoftRas-style color renderer).

Algorithm (per pixel p, over faces f):
  score(f,p) = min(BIG*w0, BIG*w1, BIG*w2, -depth)   (affine banks in px,py)
  winner(p)  = argmax_f score;  valid(p) = maxscore > THRESH
  color(p)   = winner's affine color eval, 0 if invalid.

v2: tile-culled rasterization.  The 256x256 screen is cut into 16x8-pixel
tiles (= one 128-partition chunk each).  The host culls each tile's face
list with an exact triangle-vs-tile-corner test (~6x fewer pixel-face
pairs than dense).  Tiles are sorted by face count and dealt round-robin
to the 8 cores so every core runs the identical instruction schedule
(SPMD) with per-slot face counts baked in at compile time.

Per slot: one K=36 block-diagonal bf16 matmul per 64-face block computes
all four affine banks [w0|w1|w2|negd] into PSUM; ACT drains w0/w2 to
SBUF; DVE does two mins, then one tensor_tensor_reduce fusing the final
min with the per-pixel max (z-winner) accumulation; max_index finds the
winning face; an indirect DMA gathers the winner's color coefficients,
evaluated as A*px+B*py+C with per-partition constant pixel coords.
"""

import numpy as np
import ml_dtypes

IMAGE_SIZE = 256
ORIG_SIZE = 512
DENOM_EPS = 1e-8

BIG = 1e14
THRESH = -5000.0
BAD = -3.0e30
NINIT = -3.0e38
NCORES = 8
TW, TH = 16, 8          # tile = 16x8 pixels = 128 partitions
FBLK = 128              # faces per matmul block (4*128 = 512 psum cols = 1 bank)

bf16 = ml_dtypes.bfloat16

_PROGRAM_CACHE = {}


# ----------------------------------------------------------------------------
# Host-side math (projection, coefficients)
# ----------------------------------------------------------------------------

def _project_f32(vertices, K, R, t, dist_coeffs):
    """Faithful float32 replication of the reference projection."""
    f32 = np.float32
    EPS = f32(1e-9)
    v = np.einsum('bij,bvj->bvi', R.astype(f32), vertices.astype(f32)).astype(f32) + t.astype(f32)
    x, y, z = v[..., 0], v[..., 1], v[..., 2]
    x_ = (x / (z + EPS)).astype(f32)
    y_ = (y / (z + EPS)).astype(f32)
    r2 = (x_ * x_ + y_ * y_).astype(f32)
    d = dist_coeffs.astype(f32)
    k1 = d[:, 0:1]; k2 = d[:, 1:2]; p1 = d[:, 2:3]; p2 = d[:, 3:4]; k3 = d[:, 4:5]
    radial = (f32(1.0) + k1 * r2 + k2 * r2 ** 2 + k3 * r2 ** 3).astype(f32)
    x__ = (x_ * radial + f32(2.0) * p1 * x_ * y_ + p2 * (r2 + f32(2.0) * x_ * x_)).astype(f32)
    y__ = (y_ * radial + p1 * (r2 + f32(2.0) * y_ * y_) + f32(2.0) * p2 * x_ * y_).astype(f32)
    ones = np.ones_like(x__)
    uv = np.einsum('bij,bvj->bvi', K.astype(f32),
                   np.stack([x__, y__, ones], -1).astype(f32)).astype(f32)
    OS = f32(ORIG_SIZE)
    u = (f32(2.0) * (uv[..., 0] - OS / 2) / OS).astype(f32)
    vv = (f32(2.0) * ((OS - uv[..., 1]) - OS / 2) / OS).astype(f32)
    return np.stack([u, vv, z], -1).astype(f32)          # [B,V,3]


def _face_vertices_f32(verts, faces):
    f32 = np.float32
    IM = f32(IMAGE_SIZE)
    fv = verts[0][faces[0]]                               # [F,3,3]
    fv = fv * np.array([1.0, -1.0, 1.0], dtype=f32)
    fv = (fv * (IM / 2) + IM / 2).astype(f32)
    return fv


def _build_coeffs(fv):
    """Per-face f64 affine coefficients for w0,w1,w2,negdepth + ok mask."""
    f = fv.astype(np.float64)
    x0, y0, z0 = f[:, 0, 0], f[:, 0, 1], f[:, 0, 2]
    x1, y1, z1 = f[:, 1, 0], f[:, 1, 1], f[:, 1, 2]
    x2, y2, z2 = f[:, 2, 0], f[:, 2, 1], f[:, 2, 2]
    denom_f32 = ((fv[:, 1, 1] - fv[:, 2, 1]) * (fv[:, 0, 0] - fv[:, 2, 0])
                 + (fv[:, 2, 0] - fv[:, 1, 0]) * (fv[:, 0, 1] - fv[:, 2, 1])).astype(np.float32)
    ok = np.abs(denom_f32) > np.float32(DENOM_EPS)
    d = np.where(ok, (y1 - y2) * (x0 - x2) + (x2 - x1) * (y0 - y2), 1.0)
    a0 = (y1 - y2) / d; b0 = (x2 - x1) / d
    c0 = (-(y1 - y2) * x2 - (x2 - x1) * y2) / d
    a1 = (y2 - y0) / d; b1 = (x0 - x2) / d
    c1 = (-(y2 - y0) * x2 - (x0 - x2) * y2) / d
    a2 = (y0 - y1) / d; b2 = (x1 - x0) / d
    c2 = (-(y0 - y1) * x1 - (x1 - x0) * y1) / d
    and_ = -(a0 * z0 + a1 * z1 + a2 * z2)
    bnd = -(b0 * z0 + b1 * z1 + b2 * z2)
    cnd = -(c0 * z0 + c1 * z1 + c2 * z2)
    return dict(ok=ok, w0=(a0, b0, c0), w1=(a1, b1, c1), w2=(a2, b2, c2),
                nd=(and_, bnd, cnd))


def _split3_bf16(a):
    h = a.astype(bf16)
    r1 = a - h.astype(np.float64)
    m = r1.astype(bf16)
    l = (r1 - m.astype(np.float64)).astype(bf16)
    return h, m, l


def _bank_rows9(a, b, cc):
    """9 bf16 coefficient rows for one bank (c already recentered)."""
    ah, am, al = _split3_bf16(a)
    bh, bm, bl = _split3_bf16(b)
    ch, cm, cl = _split3_bf16(cc)
    return np.stack([ah, am, al, bh, bm, bl, ch, cm, cl], 0)   # [9, n] bf16


def _cull_tiles(fv, ok):
    """Exact-corner conservative cull: per 16x8 tile, faces overlapping it.

    Returns (tiles, grid) where tiles is a list of (ty, tx, face_idx array)
    for non-empty tiles and grid holds the pixel-space placement info."""
    F = fv.shape[0]
    fxmin = fv[:, :, 0].min(1); fxmax = fv[:, :, 0].max(1)
    fymin = fv[:, :, 1].min(1); fymax = fv[:, :, 1].max(1)
    xmin, xmax = fv[:, :, 0].min(), fv[:, :, 0].max()
    ymin, ymax = fv[:, :, 1].min(), fv[:, :, 1].max()
    c_lo = max(0, int(np.floor(xmin - 0.5)) - 1)
    c_hi = min(IMAGE_SIZE - 1, int(np.ceil(xmax - 0.5)) + 1)
    r_lo = max(0, int(np.floor(ymin - 0.5)) - 1)
    r_hi = min(IMAGE_SIZE - 1, int(np.ceil(ymax - 0.5)) + 1)
    if c_hi < c_lo or r_hi < r_lo:
        return [], None
    ntx = -(-(c_hi - c_lo + 1) // TW)
    nty = -(-(r_hi - r_lo + 1) // TH)
    f64 = fv.astype(np.float64)
    okidx = np.where(ok)[0]
    tiles = []
    for ty in range(nty):
        for tx in range(ntx):
            x0 = c_lo + tx * TW + 0.5; x1 = x0 + TW - 1
            y0 = r_lo + ty * TH + 0.5; y1 = y0 + TH - 1
            m = ((fxmax[okidx] >= x0) & (fxmin[okidx] <= x1)
                 & (fymax[okidx] >= y0) & (fymin[okidx] <= y1))
            idx = okidx[m]
            if len(idx) == 0:
                continue
            v = f64[idx]
            keep = np.ones(len(idx), bool)
            corners = np.array([[x0, y0], [x0, y1], [x1, y0], [x1, y1]])
            for e in range(3):
                a = v[:, e, :2]; b = v[:, (e + 1) % 3, :2]; c3 = v[:, (e + 2) % 3, :2]
                ex = b[:, 0] - a[:, 0]; ey = b[:, 1] - a[:, 1]
                win = ex * (c3[:, 1] - a[:, 1]) - ey * (c3[:, 0] - a[:, 0])
                wc = (ex[:, None] * (corners[None, :, 1] - a[:, None, 1])
                      - ey[:, None] * (corners[None, :, 0] - a[:, None, 0]))
                allout = np.all(wc * np.sign(win)[:, None] < -1e-9, axis=1)
                keep &= ~allout
            idx = idx[keep]
            if len(idx):
                tiles.append((ty, tx, idx))
    grid = dict(c_lo=c_lo, r_lo=r_lo, ntx=ntx, nty=nty)
    return tiles, grid


# ----------------------------------------------------------------------------
# Bass program
# ----------------------------------------------------------------------------

def _build_program(cpads):
    """cpads: tuple of per-slot padded face counts (each a multiple of FBLK)."""
    import concourse.bacc as bacc
    import concourse.tile as tile
    import concourse.bass as bass
    from concourse import mybir
    from contextlib import ExitStack

    S = len(cpads)
    TOTC = sum(4 * c for c in cpads)          # total coef columns
    dt = mybir.dt
    op = mybir.AluOpType
    nc = bacc.Bacc("TRN2", target_bir_lowering=False, debug=False,
                   num_devices=NCORES)

    pixlhs = nc.dram_tensor("pixlhs", [36, 128], dt.bfloat16, kind="ExternalInput")
    coefs = nc.dram_tensor("coefs", [36, TOTC], dt.bfloat16, kind="ExternalInput")
    pxc_d = nc.dram_tensor("pxc", [128, S], dt.float32, kind="ExternalInput")
    pyc_d = nc.dram_tensor("pyc", [128, S], dt.float32, kind="ExternalInput")
    colcos = [nc.dram_tensor(f"colco{s}", [cpads[s], 16], dt.float32,
                             kind="ExternalInput") for s in range(S)]
    colout = nc.dram_tensor("colout", [128, 3 * S], dt.float32, kind="ExternalOutput")

    with tile.TileContext(nc) as tc, ExitStack() as ctx:
        const = ctx.enter_context(tc.tile_pool(name="const", bufs=1))
        psum = ctx.enter_context(tc.tile_pool(name="psum", bufs=2, space="PSUM"))
        drain = ctx.enter_context(tc.tile_pool(name="drain", bufs=2))
        work = ctx.enter_context(tc.tile_pool(name="work", bufs=2))
        scorep = ctx.enter_context(tc.tile_pool(name="scorep", bufs=2))
        idxp = ctx.enter_context(tc.tile_pool(name="idxp", bufs=3))
        accp = ctx.enter_context(tc.tile_pool(name="accp", bufs=1))

        pix_sb = const.tile([36, 128], dt.bfloat16)
        nc.sync.dma_start(out=pix_sb[:], in_=pixlhs[:])
        coef_sb = const.tile([36, TOTC], dt.bfloat16)
        pxc = const.tile([128, S], dt.float32)
        nc.sync.dma_start(out=pxc[:], in_=pxc_d[:])
        pyc = const.tile([128, S], dt.float32)
        nc.sync.dma_start(out=pyc[:], in_=pyc_d[:])

        cmx = accp.tile([128, S], dt.float32)
        gath = accp.tile([128, S, 16], dt.float32)

        # per-slot coefficient DMAs (pipelined with compute)
        off = 0
        offs = []
        for s in range(S):
            w = 4 * cpads[s]
            nc.sync.dma_start(out=coef_sb[:, off:off + w], in_=coefs[:, off:off + w])
            offs.append(off)
            off += w

        for s in range(S):
            c = cpads[s]
            nblk = c // FBLK
            off = offs[s]
            P = psum.tile([128, 2048], dt.float32, tag="P")
            for b in range(nblk):
                nc.tensor.matmul(
                    P[:, 512 * b:512 * (b + 1)],
                    pix_sb[:, :],
                    coef_sb[:, off + 512 * b: off + 512 * (b + 1)],
                    start=True, stop=True,
                )
            # ACT: drain all 4 banks PSUM -> SBUF, bank-major contiguous
            # P block layout: [w0(64)|w1(64)|w2(64)|negd(64)] per 256-col block
            # s_all layout:   [w0(c) | w1(c) | w2(c) | negd(c)]
            s_all = drain.tile([128, 2048], dt.float32, tag="s_all")
            apv = P[:, 0:1]
            in4 = bass.AP(tensor=apv.tensor, offset=apv.offset,
                          ap=[apv.ap[0], [128, 4], [512, nblk], [1, 128]])
            apo = s_all[:, 0:1]
            out4 = bass.AP(tensor=apo.tensor, offset=apo.offset,
                           ap=[apo.ap[0], [c, 4], [128, nblk], [1, 128]])
            nc.scalar.copy(out=out4, in_=in4)

            x = work.tile([128, 512], dt.float32, tag="x")
            y = work.tile([128, 512], dt.float32, tag="y")
            nc.vector.tensor_tensor(out=x[:, 0:c], in0=s_all[:, 0:c],
                                    in1=s_all[:, c:2 * c], op=op.min)
            nc.vector.tensor_tensor(out=y[:, 0:c], in0=x[:, 0:c],
                                    in1=s_all[:, 2 * c:3 * c], op=op.min)
            score = scorep.tile([128, 512], dt.float32, tag="score")
            nc.vector.tensor_tensor(out=score[:, 0:c], in0=y[:, 0:c],
                                    in1=s_all[:, 3 * c:4 * c], op=op.min)
            nc.vector.tensor_reduce(out=cmx[:, s:s + 1], in_=score[:, 0:c],
                                    axis=mybir.AxisListType.X, op=op.max)
            # argmax via max_index (in_max = 8-wide broadcast of cmx col s)
            cm_ap = cmx[:, s:s + 1]
            cm8 = bass.AP(tensor=cm_ap.tensor, offset=cm_ap.offset,
                          ap=[cm_ap.ap[0], [0, 8]])
            idx8 = idxp.tile([128, 8], dt.uint32, tag="idx8")
            nc.vector.max_index(out=idx8[:], in_max=cm8, in_values=score[:, 0:c])
            nc.gpsimd.indirect_dma_start(
                out=gath[:, s, :], out_offset=None,
                in_=colcos[s][:],
                in_offset=bass.IndirectOffsetOnAxis(ap=idx8[:, 0:1], axis=0),
                bounds_check=c - 1, oob_is_err=False,
            )

        # validity + color eval:  color_ch = (A*px + B*py + C) * vmask
        vmask = accp.tile([128, S], dt.float32)
        nc.vector.tensor_scalar(out=vmask[:], in0=cmx[:], scalar1=float(THRESH),
                                scalar2=None, op0=op.is_gt)
        cout = accp.tile([128, 3 * S], dt.float32)
        for ch in range(3):
            def gv(k):
                apv = gath[:, 0, 0:1]
                return bass.AP(tensor=apv.tensor, offset=apv.offset + 3 * ch + k,
                               ap=[apv.ap[0], [16, S]])
            t1 = work.tile([128, S], dt.float32, tag="t1")
            t2 = work.tile([128, S], dt.float32, tag="t2")
            nc.vector.tensor_tensor(out=t1[:], in0=gv(0), in1=pxc[:], op=op.mult)
            nc.vector.tensor_tensor(out=t2[:], in0=gv(1), in1=pyc[:], op=op.mult)
            nc.vector.tensor_tensor(out=t1[:], in0=t1[:], in1=t2[:], op=op.add)
            nc.vector.tensor_tensor(out=t1[:], in0=t1[:], in1=gv(2), op=op.add)
            nc.vector.tensor_tensor(out=cout[:, ch * S:(ch + 1) * S], in0=t1[:],
                                    in1=vmask[:], op=op.mult)
        nc.sync.dma_start(out=colout[:], in_=cout[:])

    nc.compile()
    return nc


def _get_program(cpads):
    key = tuple(cpads)
    if key not in _PROGRAM_CACHE:
        _PROGRAM_CACHE[key] = _build_program(key)
    return _PROGRAM_CACHE[key]


# ----------------------------------------------------------------------------
# Host orchestration
# ----------------------------------------------------------------------------

def prepare(vertices, faces, textures, K, R, t, dist_coeffs):
    """All host-side prep.  Returns (cpads, in_maps, scatter)."""
    verts = _project_f32(np.asarray(vertices), np.asarray(K), np.asarray(R),
                         np.asarray(t), np.asarray(dist_coeffs))
    fv = _face_vertices_f32(verts, np.asarray(faces))
    co = _build_coeffs(fv)
    tiles, grid = _cull_tiles(fv, co['ok'])
    if not tiles:
        return None
    tex = np.asarray(textures)[0].astype(np.float64)      # [F,3,C]

    # color affine coefficients per face (global coords)  [F, 9] f64
    F = fv.shape[0]
    colABC = np.zeros((F, 9), dtype=np.float64)
    for ch in range(3):
        t0, t1, t2 = tex[:, 0, ch], tex[:, 1, ch], tex[:, 2, ch]
        colABC[:, 3 * ch + 0] = co['w0'][0] * t0 + co['w1'][0] * t1 + co['w2'][0] * t2
        colABC[:, 3 * ch + 1] = co['w0'][1] * t0 + co['w1'][1] * t1 + co['w2'][1] * t2
        colABC[:, 3 * ch + 2] = co['w0'][2] * t0 + co['w1'][2] * t1 + co['w2'][2] * t2

    # sort tiles by count desc, deal round-robin; slot shape = octet max
    tiles.sort(key=lambda tt: -len(tt[2]))
    ntile = len(tiles)
    S = -(-ntile // NCORES)
    cpads = []
    for s in range(S):
        grp = tiles[8 * s: 8 * s + 8]
        cmax = max(len(tt[2]) for tt in grp)
        cpads.append(max(FBLK, -(-cmax // FBLK) * FBLK))
    TOTC = sum(4 * c for c in cpads)

    # per-partition local pixel coords (tile-local, constant across slots)
    pp = np.arange(128)
    pxl = (pp % TW) - (TW / 2 - 0.5)          # -7.5 .. 7.5
    pyl = (pp // TW) - (TH / 2 - 0.5)         # -3.5 .. 3.5
    pxc = np.repeat(pxl.astype(np.float32).reshape(128, 1), S, axis=1)
    pyc = np.repeat(pyl.astype(np.float32).reshape(128, 1), S, axis=1)

    # stationary matmul operand [36,128]: 4 banks x [px*3, py*3, 1*3]
    pixlhs = np.zeros((36, 128), dtype=bf16)
    rows9 = [pxl, pxl, pxl, pyl, pyl, pyl, np.ones(128), np.ones(128), np.ones(128)]
    for g in range(4):
        for r in range(9):
            pixlhs[9 * g + r, :] = rows9[r].astype(bf16)
    assert np.all(pixlhs[0].astype(np.float64) == pxl)
    assert np.all(pixlhs[3].astype(np.float64) == pyl)

    c_lo, r_lo = grid['c_lo'], grid['r_lo']
    banks = ['w0', 'w1', 'w2']

    in_maps = []
    rows_of = np.zeros((NCORES, S, 128), dtype=np.int32)
    cols_of = np.zeros((NCORES, S, 128), dtype=np.int32)
    real_of = np.zeros((NCORES, S, 128), dtype=bool)
    for k in range(NCORES):
        coefs = np.zeros((36, TOTC), dtype=bf16)
        colco_s = [np.zeros((cpads[s], 16), dtype=np.float32) for s in range(S)]
        off = 0
        for s in range(S):
            c = cpads[s]
            ti = 8 * s + k
            if ti < ntile:
                ty, tx, fidx = tiles[ti]
                n = len(fidx)
                # tile center (pixel-center coords): local px = gx+0.5 - sx
                sx = c_lo + tx * TW + TW / 2.0
                sy = r_lo + ty * TH + TH / 2.0
                gx = c_lo + tx * TW + (pp % TW)
                gy = r_lo + ty * TH + (pp // TW)
                real = (gx <= IMAGE_SIZE - 1) & (gy <= IMAGE_SIZE - 1)
                rows_of[k, s] = np.minimum(gy, IMAGE_SIZE - 1)
                cols_of[k, s] = np.minimum(gx, IMAGE_SIZE - 1)
                real_of[k, s] = real
            else:
                n = 0
            # coefficient columns, block-diagonal per 64-face block
            for g in range(4):
                if g < 3:
                    a, b, cc = (v.copy() for v in co[banks[g]])
                    a *= BIG; b *= BIG; cc *= BIG
                else:
                    a, b, cc = (v.copy() for v in co['nd'])
                if n:
                    av = a[fidx]; bv = b[fidx]; cv = cc[fidx] + av * sx + bv * sy
                av = np.concatenate([av, np.zeros(c - n)]) if n else np.zeros(c)
                bv = np.concatenate([bv, np.zeros(c - n)]) if n else np.zeros(c)
                pad_c = np.full(c - n, BAD if g == 3 else 0.0)
                cv = np.concatenate([cv, pad_c]) if n else np.full(c, BAD if g == 3 else 0.0)
                r9 = _bank_rows9(av, bv, cv)                  # [9, c]
                for b_ in range(c // FBLK):
                    coefs[9 * g:9 * g + 9,
                          off + 512 * b_ + 128 * g: off + 512 * b_ + 128 * (g + 1)] = \
                        r9[:, 128 * b_:128 * (b_ + 1)]
            if n:
                ABC = colABC[fidx]                            # [n, 9]
                ABCr = ABC.copy()
                for ch in range(3):
                    ABCr[:, 3 * ch + 2] += ABC[:, 3 * ch + 0] * sx + ABC[:, 3 * ch + 1] * sy
                colco_s[s][0:n, 0:9] = ABCr.astype(np.float32)
            off += 4 * c
        im = dict(pixlhs=pixlhs, coefs=coefs, pxc=pxc, pyc=pyc)
        for s in range(S):
            im[f"colco{s}"] = colco_s[s]
        in_maps.append(im)

    scatter = dict(rows_of=rows_of, cols_of=cols_of, real_of=real_of,
                   S=S, ntile=ntile)
    return cpads, in_maps, scatter


def assemble(results, scatter):
    out = np.zeros((1, 3, IMAGE_SIZE, IMAGE_SIZE), dtype=np.float32)
    S = scatter['S']
    for k in range(NCORES):
        col = results[k]['colout'].reshape(128, 3, S)      # [128, 3, S]
        for s in range(S):
            if 8 * s + k >= scatter['ntile']:
                continue
            real = scatter['real_of'][k, s]
            rr = scatter['rows_of'][k, s][real]
            cc = scatter['cols_of'][k, s][real]
            out[0, :, rr, cc] = col[real, :, s]
    return out


def kernel(**inputs):
    from concourse.bass_utils import run_bass_kernel_spmd

    prep = prepare(**inputs)
    if prep is None:
        return np.zeros((1, 3, IMAGE_SIZE, IMAGE_SIZE), dtype=np.float32)
    cpads, in_maps, scatter = prep
    nc = _get_program(cpads)
    res = run_bass_kernel_spmd(nc, in_maps, core_ids=list(range(NCORES)))
    return assemble(res.results, scatter)


if __name__ == "__main__":
    pass
